# revision 5
# baseline (speedup 1.0000x reference)
"""Trainium2 Bass kernel for nn_ContrastiveLearningLoss.

Strategy (data-parallel over the flattened region axis N = max_num*B = 40):
  - Each of 8 cores gets 5 regions: slabs of features_q/features_k reshaped
    to (40, 256, 11264) and the mask reshaped to (40, 11264) (flatten orders
    intentionally differ in the reference, but both are plain reshapes of
    their own tensors, so pairing region n of each is exactly the reference
    pairing).
  - On-device, each core computes masked sums s[n, c] = sum_hw f[n,c,hw]*m[n,hw]
    for q and k.  The mask row of a region is loaded ONCE as a bf16 (1, HW)
    strip (bf16 is exact for 0/1), and the otherwise-idle PE broadcasts it to
    all 128 partitions chunk by chunk via an outer product ones(1,128)^T @
    mrow(1,512) into PSUM — this keeps the 128x broadcast amplification off
    the DMA engines, which then carry nothing but the mandatory feature reads.
  - The masked multiply+reduce is a single fused DVE scalar_tensor_tensor
    (out = (f*1)*mask_psum, accum_out = row sums) per (stream, chunk).
  - This walrus build allows at most ONE sync-wait per compute instruction, so
    tiny per-bank "guard" copies on DVE absorb the PE-matmul dependencies
    before each chunk's 4 STT ops; each STT then only waits on its own
    feature DMA.
  - The tiny (40, 256) epilogue (means, normalize, 40x40 similarity, CE)
    runs on host in float32.
"""

import ml_dtypes
import numpy as np

MAX_NUM, B, C, H, W = 10, 4, 256, 64, 176
HW = H * W          # 11264
N = MAX_NUM * B     # 40
N_CORES = 8
R = N // N_CORES    # 5 regions per core
TAU = 0.07
EPS = 1e-12

# per-region hw chunks; each chunk is one f-tile group and one PSUM mask tile
CW = 2048
FGROUPS = [(0, 2048), (2048, 2048), (4096, 2048), (6144, 2048),
           (8192, 2048), (10240, 1024)]
N_CHUNKS = len(FGROUPS)
BANK = 512          # PSUM bank width in f32; one matmul writes one bank

_CACHE = {}


def _split_multi_waits(bir_bytes):
    """Legalize the BIR for this walrus build, which encodes at most ONE
    sync-wait per instruction: any instruction carrying N>1 waits gets N-1
    preceding same-engine Drain carriers, one wait each (same semantics —
    the engine executes them in order before the instruction)."""
    import json

    m = json.loads(bir_bytes)
    k = 0
    for fn in m["functions"]:
        for bb in fn["blocks"]:
            out = []
            for inst in bb["instructions"]:
                si = inst.get("sync_info")
                waits = (si or {}).get("on_wait") or []
                if len(waits) > 1:
                    for w in waits[:-1]:
                        k += 1
                        carrier = {
                            "engine": inst["engine"],
                            "ins": [],
                            "outs": [],
                            "name": f"{inst['name']}-sw{k}",
                            "opcode": "Drain",
                            "sync_info": {"on_update": [], "on_wait": [w]},
                        }
                        if "debug" in inst:
                            carrier["debug"] = inst["debug"]
                        out.append(carrier)
                    si["on_wait"] = [waits[-1]]
                out.append(inst)
            bb["instructions"] = out
    return json.dumps(m).encode()


def _build_bass():
    import concourse.bass as bass
    import concourse.tile as tile
    from concourse import mybir

    nc = bass.Bass(trn_type="TRN2")
    f32 = mybir.dt.float32
    bf16 = mybir.dt.bfloat16
    fq = nc.dram_tensor("fq", (R, C, HW), f32, kind="ExternalInput")
    fk = nc.dram_tensor("fk", (R, C, HW), f32, kind="ExternalInput")
    mk = nc.dram_tensor("mask", (R, HW), bf16, kind="ExternalInput")
    out = nc.dram_tensor("out", (128, R * 4 * N_CHUNKS), f32, kind="ExternalOutput")

    with tile.TileContext(nc) as tc:
        with (
            tc.tile_pool(name="singles", bufs=1) as singles,
            tc.tile_pool(name="fpool", bufs=3) as fpool,
            tc.tile_pool(name="mrows", bufs=2) as mrows,
            tc.tile_pool(name="mpsum", bufs=2, space="PSUM") as mpsum,
        ):
            junk = singles.tile([128, R * N_CHUNKS * 4], f32, tag="junk")
            acc = singles.tile([128, R * 4 * N_CHUNKS], f32, tag="acc")
            ones = singles.tile([1, 128], bf16, tag="ones")
            nc.gpsimd.memset(ones[:, :], 1.0)

            srcs = [(fq, 0), (fq, 1), (fk, 0), (fk, 1)]
            for r in range(R):
                mrow = mrows.tile([1, HW], bf16, tag="mrow", name="mrow")
                nc.sync.dma_start(out=mrow[:, :], in_=mk[r:r + 1, :])
                for g, (goff, gw) in enumerate(FGROUPS):
                    nb = gw // BANK
                    ps = mpsum.tile([128, CW], f32, tag="ps", name="ps")
                    for k in range(nb):
                        # outer product: broadcast 512 mask values to all 128
                        # partitions; one PSUM bank per matmul.
                        nc.tensor.matmul(
                            out=ps[:, k * BANK:(k + 1) * BANK],
                            lhsT=ones[:, :],
                            rhs=mrow[:, goff + k * BANK:goff + (k + 1) * BANK],
                            start=True,
                            stop=True,
                        )
                    gidx = (r * N_CHUNKS + g) * 4
                    for k in range(nb):
                        # guard: absorb matmul k's dependency into DVE's clock
                        # so each STT below carries only its own f-DMA wait.
                        nc.vector.tensor_copy(
                            out=junk[:, gidx + k:gidx + k + 1],
                            in_=ps[:, k * BANK:k * BANK + 1],
                        )
                    for s, (src, half) in enumerate(srcs):
                        ft = fpool.tile([128, CW], f32, tag=f"f{s}", name=f"ft{s}")
                        nc.sync.dma_start(
                            out=ft[:, :gw],
                            in_=src[r, half * 128:(half + 1) * 128, goff:goff + gw],
                        )
                        col = (r * 4 + s) * N_CHUNKS + g
                        # out is written in-place into the f tile: its last
                        # writer is the same DMA the STT already waits on, so
                        # no extra WAW wait is generated (1-wait limit).
                        nc.vector.scalar_tensor_tensor(
                            out=ft[:, :gw],
                            in0=ft[:, :gw],
                            scalar=1.0,
                            in1=ps[:, :gw],
                            op0=mybir.AluOpType.mult,
                            op1=mybir.AluOpType.mult,
                            accum_out=acc[:, col:col + 1],
                        )
            nc.sync.dma_start(out=out[:, :], in_=acc[:, :])

    orig_to_json = nc.to_json_bytes
    nc.to_json_bytes = lambda: _split_multi_waits(orig_to_json())
    return nc


def _get_bass():
    if "nc" not in _CACHE:
        _CACHE["nc"] = _build_bass()
    return _CACHE["nc"]


def _device_masked_sums(fq40, fk40, mk40, trace=False):
    """fq40/fk40: (40, 256, 11264) f32; mk40: (40, 11264) bf16.
    Returns sums_q, sums_k each (40, 256) f32 (and the run result object)."""
    from concourse.bass_utils import run_bass_kernel_spmd

    nc = _get_bass()
    in_maps = []
    for i in range(N_CORES):
        sl = slice(i * R, (i + 1) * R)
        in_maps.append({
            "fq": np.ascontiguousarray(fq40[sl]),
            "fk": np.ascontiguousarray(fk40[sl]),
            "mask": np.ascontiguousarray(mk40[sl]),
        })
    res = run_bass_kernel_spmd(nc, in_maps, core_ids=list(range(N_CORES)), trace=trace)
    sums_q = np.empty((N, C), dtype=np.float32)
    sums_k = np.empty((N, C), dtype=np.float32)
    for i, r in enumerate(res.results):
        o = r["out"].reshape(128, R, 4, N_CHUNKS).sum(axis=3, dtype=np.float32)
        for rr in range(R):
            n = i * R + rr
            sums_q[n, 0:128] = o[:, rr, 0]
            sums_q[n, 128:256] = o[:, rr, 1]
            sums_k[n, 0:128] = o[:, rr, 2]
            sums_k[n, 128:256] = o[:, rr, 3]
    return sums_q, sums_k, res


def _epilogue(sums_q, sums_k, cnt):
    mean_q = sums_q / cnt[:, None]
    mean_k = sums_k / cnt[:, None]
    pad = mean_k[:, 0] != 0

    nrm_q = np.maximum(np.linalg.norm(mean_q, axis=-1, keepdims=True), EPS).astype(np.float32)
    nrm_k = np.maximum(np.linalg.norm(mean_k, axis=-1, keepdims=True), EPS).astype(np.float32)
    nq = mean_q / nrm_q
    nk = mean_k / nrm_k

    sim = (nk @ nq.T).astype(np.float32)
    logits = sim / np.float32(TAU)
    m = logits.max(axis=-1, keepdims=True)
    lse = np.log(np.exp(logits - m).sum(axis=-1, keepdims=True)).astype(np.float32) + m
    logp = logits - lse
    ce = -logp[np.arange(N), np.arange(N)]
    padf = pad.astype(np.float32)
    loss = (ce * padf).sum() / padf.sum()
    return np.asarray(loss, dtype=np.float32)


def kernel(features_q, features_k, mask, _trace=False, _ret_res=False):
    fq40 = np.asarray(features_q, dtype=np.float32).reshape(N, C, HW)
    fk40 = np.asarray(features_k, dtype=np.float32).reshape(N, C, HW)
    mku8 = np.asarray(mask).astype(np.uint8).reshape(N, HW)
    mk40 = mku8.astype(ml_dtypes.bfloat16)

    sums_q, sums_k, res = _device_masked_sums(fq40, fk40, mk40, trace=_trace)
    cnt = np.maximum(mku8.sum(axis=1, dtype=np.int64).astype(np.float32), np.float32(1.0))
    loss = _epilogue(sums_q, sums_k, cnt)
    if _ret_res:
        return loss, res
    return loss
